# revision 1
# baseline (speedup 1.0000x reference)
"""Causal self-attention (B=2, S=2048, D=2048, H=16, HD=128) on 8 TRN2 cores.

Sharding: core c -> batch b = c//4, heads 4*(c%4)..4*(c%4)+3 (tensor-parallel
over heads within a batch; data-parallel over batch across core groups).
Each core:
  phase 1: Q^T/K^T (RoPE applied) and V projections for its 4 heads, streamed
           over s-blocks of 512, bounced to DRAM scratch.
  phase 2: causal attention per head in transposed-score layout
           (S^T[k,q] tiles), softmax without max-subtraction, row-sums via a
           ones-vector matmul, PV produces ctx^T[hd,q] directly.
  phase 3: partial output projection ctx^T.T @ Wo_rows -> [2048, 2048].
Host sums the 4 partials per batch.

All tensors feeding the PE are float32r (FP22 multiply, FP32 accumulate) for
full PE rate. Emission is software-pipelined so the PE never sits behind a
cross-engine (ACT/DVE) round trip: l/PV matmuls lag the score matmuls by 2
chunks, the RoPE shuffle matmul lags the projection group by 1, and each
q-block's softmax-normalization chain is emitted inside the next q-block's
chunk stream.
"""

import math
from collections import deque

import numpy as np

import concourse.bacc as bacc
import concourse.mybir as mybir
from concourse.tile import TileContext
from concourse.bass_utils import run_bass_kernel_spmd

B, S, D = 2, 2048, 2048
H, HD = 16, 128
ROPE_THETA = 10000.0

N_CORES = 8
CORES_PER_BATCH = 4
HPC = H // (N_CORES // B)  # heads per core = 4
HL = HPC * HD              # 512 local head-dim columns
NDC = D // 128             # 16 contraction chunks
NSB = S // 512             # 4 s-blocks
NKC = S // 128             # 16 k-chunks

F32 = mybir.dt.float32
F32R = mybir.dt.float32r
AF = mybir.ActivationFunctionType

USE_F32R = True
# dtype for every tensor that feeds the PE: FP22 multiply at full rate
FMM = F32R if USE_F32R else F32


def _mm(nc, out, lhsT, rhs, start, stop):
    nc.tensor.matmul(out, lhsT, rhs, start=start, stop=stop)


def _build():
    nc = bacc.Bacc("TRN2", target_bir_lowering=False, debug=False)

    xT = nc.dram_tensor("xT", [D, S], FMM, kind="ExternalInput")
    wq = nc.dram_tensor("wq", [D, HL], FMM, kind="ExternalInput")
    wk = nc.dram_tensor("wk", [D, HL], FMM, kind="ExternalInput")
    wv = nc.dram_tensor("wv", [D, HL], FMM, kind="ExternalInput")
    wo = nc.dram_tensor("wo", [HL, D], FMM, kind="ExternalInput")
    cosT = nc.dram_tensor("cosT", [HD, S], F32, kind="ExternalInput")
    sinT = nc.dram_tensor("sinT", [HD, S], F32, kind="ExternalInput")
    pmatT = nc.dram_tensor("pmatT", [HD, HD], FMM, kind="ExternalInput")
    maskT = nc.dram_tensor("maskT", [128, 512], F32, kind="ExternalInput")
    onesd = nc.dram_tensor("onesd", [128, 128], FMM, kind="ExternalInput")
    out = nc.dram_tensor("out", [S, D], F32, kind="ExternalOutput")

    with TileContext(nc) as tc:
        with (
            tc.tile_pool(name="dram", bufs=1, space="DRAM") as dpool,
            tc.tile_pool(name="consts", bufs=1) as consts,
            tc.tile_pool(name="psA", bufs=3, space="PSUM") as psA,
            tc.tile_pool(name="psB", bufs=3, space="PSUM") as psB,
            tc.tile_pool(name="psC", bufs=2, space="PSUM") as psC,
        ):
            qTs = dpool.tile([HPC, HD, S], FMM, name="qTs")
            kTs = dpool.tile([HPC, HD, S], FMM, name="kTs")
            vs = dpool.tile([S, HL], FMM, name="vs")

            pmat_sb = consts.tile([HD, HD], FMM, name="pmat_sb")
            mask_sb = consts.tile([128, 512], F32, name="mask_sb")
            ones_sb = consts.tile([128, 128], FMM, name="ones_sb")
            ones_col = ones_sb[:, 0:1]
            ones_row = ones_sb[0:1, :]
            gpwarm = consts.tile([128, 128], F32, name="gpwarm")

            def emit_const_dmas():
                nc.sync.dma_start(out=pmat_sb[:], in_=pmatT[:])
                nc.sync.dma_start(out=mask_sb[:], in_=maskT[:])
                nc.sync.dma_start(out=ones_sb[:], in_=onesd[:])
                # warm up the GpSimd library load off the critical path
                nc.gpsimd.partition_broadcast(gpwarm[:], mask_sb[0:1, 0:128])

            # ---------------- phase 1: projections + RoPE ----------------
            with (
                tc.tile_pool(name="wpool", bufs=1) as wpool,
                tc.tile_pool(name="xtp", bufs=27) as xtp,
                tc.tile_pool(name="st1", bufs=2) as st1,
            ):
                cos_sb = wpool.tile([HD, S], F32, name="cos_sb")
                sin_sb = wpool.tile([HD, S], F32, name="sin_sb")

                def load_w_chunk(w_d, dc, tag):
                    wt = wpool.tile([128, HL], FMM, tag=f"{tag}{dc}", name="wt")
                    nc.sync.dma_start(out=wt[:], in_=w_d[dc * 128:(dc + 1) * 128, :])
                    return wt

                def load_xt_chunk(dc, sb):
                    xt = xtp.tile([128, 512], FMM, tag="xt", name="xt")
                    nc.sync.dma_start(
                        out=xt[:], in_=xT[dc * 128:(dc + 1) * 128,
                                          sb * 512:(sb + 1) * 512])
                    return xt

                # prologue: interleave first s-block's x chunks with wq chunks
                # so the first matmul group starts after ~0.5MB of DMA.
                xts = []
                wq_t, wk_t, wv_t = [], [], []
                for dc in range(NDC):
                    xts.append(load_xt_chunk(dc, 0))
                    wq_t.append(load_w_chunk(wq, dc, "wq"))
                    if dc == 3:
                        emit_const_dmas()
                        nc.sync.dma_start(out=cos_sb[:], in_=cosT[:])
                        nc.sync.dma_start(out=sin_sb[:], in_=sinT[:])
                for dc in range(NDC):
                    wk_t.append(load_w_chunk(wk, dc, "wk"))
                for dc in range(NDC):
                    wv_t.append(load_w_chunk(wv, dc, "wv"))

                # jobs: ("qk", w_sb, dst, h, sl) or ("v", sc, sb)
                finishers = deque()

                def emit_finisher():
                    kind, args = finishers.popleft()
                    if kind == "qk":
                        ps, qraw, dst, h, sl = args
                        rot = psB.tile([128, 512], F32, tag="b", name="rot")
                        _mm(nc, rot[:], pmat_sb[:], qraw[:], start=True, stop=True)
                        acos = st1.tile([128, 512], F32, tag="acos", name="acos")
                        nc.vector.tensor_mul(acos[:], ps[:], cos_sb[:, sl])
                        rsin = st1.tile([128, 512], F32, tag="rsin", name="rsin")
                        nc.vector.tensor_mul(rsin[:], rot[:], sin_sb[:, sl])
                        rope = st1.tile([128, 512], FMM, tag="rope", name="rope")
                        nc.vector.tensor_add(rope[:], rsin[:], acos[:])
                        nc.scalar.dma_start(out=dst[:, sl], in_=rope[:])
                    else:
                        ps, sb, sc = args
                        vsb = st1.tile([128, 512], FMM, tag="vsb", name="vsb")
                        nc.scalar.activation(vsb[:], ps[:], AF.Copy)
                        nc.scalar.dma_start(
                            out=vs[sb * 512 + sc * 128: sb * 512 + (sc + 1) * 128, :],
                            in_=vsb[:],
                        )

                for sb in range(NSB):
                    sl = slice(sb * 512, (sb + 1) * 512)
                    if sb > 0:
                        xts = [load_xt_chunk(dc, sb) for dc in range(NDC)]

                    for w_t, dst in ((wq_t, qTs), (wk_t, kTs)):
                        for h in range(HPC):
                            ps = psA.tile([128, 512], F32, tag="a", name="ps")
                            for dc in range(NDC):
                                _mm(nc, ps[:],
                                    w_t[dc][:, h * HD:(h + 1) * HD],
                                    xts[dc][:],
                                    start=(dc == 0), stop=(dc == NDC - 1))
                            qraw = st1.tile([128, 512], FMM, tag="qraw", name="qraw")
                            nc.scalar.activation(qraw[:], ps[:], AF.Copy)
                            finishers.append(("qk", (ps, qraw, dst[h], h, sl)))
                            if len(finishers) > 1:
                                emit_finisher()

                    for sc in range(4):
                        ps = psA.tile([128, 512], F32, tag="a", name="ps")
                        for dc in range(NDC):
                            _mm(nc, ps[:],
                                xts[dc][:, sc * 128:(sc + 1) * 128],
                                wv_t[dc][:],
                                start=(dc == 0), stop=(dc == NDC - 1))
                        finishers.append(("v", (ps, sb, sc)))
                        if len(finishers) > 1:
                            emit_finisher()
                while finishers:
                    emit_finisher()

            # ---------- phase 2+3: attention + output projection ----------
            with tc.tile_pool(name="ctxp", bufs=1) as ctxp:
                ctxs = [ctxp.tile([128, S], FMM, name=f"ctxT{h}") for h in range(HPC)]
                with (
                    tc.tile_pool(name="qkv2", bufs=3) as qkv2,
                    tc.tile_pool(name="pp", bufs=8) as pp,
                    tc.tile_pool(name="sm", bufs=3) as sm,
                    tc.tile_pool(name="wop", bufs=1) as wop,
                    tc.tile_pool(name="outp", bufs=3) as outp,
                ):
                    wo_sb = wop.tile([128, HPC * D], FMM, name="wo_sb")
                    lagq = deque()   # (lps, pv, pt, vt_chunk, ncols, first, last)
                    fin = deque()    # (h, qb, lps, pv)
                    lpv_done = set()  # ids of lps tiles whose accumulation is fully emitted

                    def emit_lpv(job):
                        lps, pv, pt, vtc, ncols, first, last = job
                        _mm(nc, lps[:, 512 - ncols:], ones_col, pt[:, :ncols],
                            start=first, stop=last)
                        _mm(nc, pv[:, 512 - ncols:], vtc, pt[:, :ncols],
                            start=first, stop=last)
                        if last:
                            lpv_done.add(id(lps))

                    def emit_finalize(job):
                        # broadcast raw l via PE (short chain), then approx
                        # reciprocal + scale on DVE, fully off the PE path
                        h, qb, lps, pv = job
                        lsb = sm.tile([1, 512], F32, tag="lsb", name="lsb")
                        nc.vector.tensor_copy(lsb[:], lps[:])
                        repsb = sm.tile([128, 512], F32, tag="repsb", name="repsb")
                        nc.gpsimd.partition_broadcast(repsb[:], lsb[:])
                        rcps = sm.tile([128, 512], F32, tag="rcps", name="rcps")
                        rcp = sm.tile([128, 512], F32, tag="rcp", name="rcp")
                        nc.vector.reciprocal_approx_accurate(rcp[:], repsb[:], rcps[:])
                        nc.vector.tensor_mul(ctxs[h][:, qb * 512:(qb + 1) * 512],
                                             pv[:], rcp[:])

                    def emit_outproj(job):
                        qb, half = job
                        for qc in range(4 * qb + 2 * half, 4 * qb + 2 * half + 2):
                            for db in range(D // 512):
                                ops = psA.tile([128, 512], F32, tag="a", name="ops")
                                for h in range(HPC):
                                    _mm(nc, ops[:],
                                        ctxs[h][:, qc * 128:(qc + 1) * 128],
                                        wo_sb[:, h * D + db * 512: h * D + (db + 1) * 512],
                                        start=(h == 0), stop=(h == HPC - 1))
                                osb = outp.tile([128, 512], F32, tag="osb", name="osb")
                                nc.scalar.activation(osb[:], ops[:], AF.Copy)
                                nc.sync.dma_start(
                                    out=out[qc * 128:(qc + 1) * 128,
                                            db * 512:(db + 1) * 512],
                                    in_=osb[:])

                    outproj_ready = [0] * NSB  # finalizes emitted per q-block
                    outproj_q = deque()        # q-blocks whose ctx is complete

                    def pop_finalize():
                        if fin and id(fin[0][2]) in lpv_done:
                            job = fin.popleft()
                            emit_finalize(job)
                            qb = job[1]
                            outproj_ready[qb] += 1
                            if outproj_ready[qb] == HPC:
                                outproj_q.append((qb, 0))
                                outproj_q.append((qb, 1))

                    vsrc = vs.rearrange("(kc p) c -> p kc c", p=128)
                    for h in range(HPC):
                        qts, kts, vts = [], [], []
                        for p in range(4):
                            cs = slice(p * 512, (p + 1) * 512)
                            ktp = qkv2.tile([128, 512], FMM, tag=f"kt{p}", name="ktp")
                            nc.sync.dma_start(out=ktp[:], in_=kTs[h][:, cs])
                            kts.append(ktp)
                            qtp = qkv2.tile([128, 512], FMM, tag=f"qt{p}", name="qtp")
                            nc.sync.dma_start(out=qtp[:], in_=qTs[h][:, cs])
                            qts.append(qtp)
                            vtp = qkv2.tile([128, 4, HD], FMM, tag=f"vt{p}", name="vtp")
                            nc.sync.dma_start(
                                out=vtp[:],
                                in_=vsrc[:, p * 4:(p + 1) * 4, h * HD:(h + 1) * HD])
                            vts.append(vtp)
                        if h == 0:
                            nc.sync.dma_start(
                                out=wo_sb[:],
                                in_=wo.rearrange("(r p) c -> p r c", p=128))

                        for qb in range(NSB):
                            nk = 4 * qb + 4
                            lps = psC.tile([1, 512], F32, tag="c", name="lps")
                            pv = psB.tile([128, 512], F32, tag="b", name="pv")
                            for kc in range(nk):
                                j = kc - 4 * qb
                                ncols = 512 if j < 0 else 512 - 128 * j
                                q0 = qb * 512 + 512 - ncols
                                sps = psA.tile([128, 512], F32, tag="a", name="sps")
                                _mm(nc, sps[:, :ncols],
                                    kts[kc // 4][:, (kc % 4) * 128:(kc % 4 + 1) * 128],
                                    qts[qb][:, q0 - qb * 512:],
                                    start=True, stop=True)
                                pt = pp.tile([128, 512], FMM, tag="pt", name="pt")
                                nc.scalar.activation(pt[:, :ncols], sps[:, :ncols], AF.Exp)
                                if j >= 0:
                                    nc.vector.tensor_mul(pt[:, :ncols], pt[:, :ncols],
                                                         mask_sb[:, :ncols])
                                lagq.append((lps, pv, pt, vts[kc // 4][:, kc % 4, :],
                                             ncols, kc == 0, kc == nk - 1))
                                while len(lagq) > 2:
                                    emit_lpv(lagq.popleft())
                                if kc % 2 == 1:
                                    if outproj_q:
                                        emit_outproj(outproj_q.popleft())
                                    else:
                                        pop_finalize()
                            fin.append((h, qb, lps, pv))
                    while lagq:
                        emit_lpv(lagq.popleft())
                    while fin:
                        pop_finalize()
                        while outproj_q:
                            emit_outproj(outproj_q.popleft())

    nc.compile()
    return nc


_NC_CACHE = None


def _get_nc():
    global _NC_CACHE
    if _NC_CACHE is None:
        _NC_CACHE = _build()
    return _NC_CACHE


def _host_tables():
    # Replicate reference RoPE tables in float32 arithmetic, transposed.
    inv_freq = np.float32(1.0) / np.power(
        np.float32(ROPE_THETA), np.arange(0, HD, 2).astype(np.float32) / np.float32(HD)
    )
    pos = np.arange(S, dtype=np.float32)
    freqs = pos[:, None] * inv_freq[None, :]
    angles = np.concatenate([freqs, freqs], axis=1)  # [S, HD]
    cos_t = np.ascontiguousarray(np.cos(angles).astype(np.float32).T)  # [HD, S]
    sin_t = np.ascontiguousarray(np.sin(angles).astype(np.float32).T)
    # rotate_half as a left-multiply matrix P: (P q)[2i] = -q[2i+1], [2i+1] = q[2i].
    # matmul computes lhsT.T @ rhs, so feed P.T.
    pmat = np.zeros((HD, HD), dtype=np.float32)
    for i in range(HD // 2):
        pmat[2 * i, 2 * i + 1] = -1.0
        pmat[2 * i + 1, 2 * i] = 1.0
    pmat_t = np.ascontiguousarray(pmat.T)
    mask = (np.arange(128)[:, None] <= np.arange(512)[None, :]).astype(np.float32)
    return cos_t, sin_t, pmat_t, mask


_ONES = np.ones((128, 128), dtype=np.float32)


def kernel(x, Wq, Wk, Wv, Wo):
    x = np.asarray(x, dtype=np.float32)
    Wq = np.asarray(Wq, dtype=np.float32)
    Wk = np.asarray(Wk, dtype=np.float32)
    Wv = np.asarray(Wv, dtype=np.float32)
    Wo = np.asarray(Wo, dtype=np.float32)

    results = _run_device(x, Wq, Wk, Wv, Wo)

    out = np.empty((B, S, D), dtype=np.float32)
    for b in range(B):
        acc = results[b * CORES_PER_BATCH]["out"]
        for i in range(1, CORES_PER_BATCH):
            acc = acc + results[b * CORES_PER_BATCH + i]["out"]
        out[b] = acc
    return out


def _make_in_maps(x, Wq, Wk, Wv, Wo):
    cos_t, sin_t, pmat_t, mask = _host_tables()
    scale = np.float32(1.0 / math.sqrt(HD))
    wq_scaled = (Wq * scale).astype(np.float32)
    xTb = [np.ascontiguousarray(x[b].T) for b in range(B)]
    in_maps = []
    for c in range(N_CORES):
        b = c // CORES_PER_BATCH
        g = c % CORES_PER_BATCH
        hs = slice(g * HL, (g + 1) * HL)
        in_maps.append({
            "xT": xTb[b],
            "wq": np.ascontiguousarray(wq_scaled[:, hs]),
            "wk": np.ascontiguousarray(Wk[:, hs]),
            "wv": np.ascontiguousarray(Wv[:, hs]),
            "wo": np.ascontiguousarray(Wo[hs, :]),
            "cosT": cos_t,
            "sinT": sin_t,
            "pmatT": pmat_t,
            "maskT": mask,
            "onesd": _ONES,
        })
    return in_maps


def _run_device(x, Wq, Wk, Wv, Wo, trace=False):
    nc = _get_nc()
    in_maps = _make_in_maps(x, Wq, Wk, Wv, Wo)
    res = run_bass_kernel_spmd(nc, in_maps, core_ids=list(range(N_CORES)), trace=trace)
    if trace:
        return res
    return res.results


def run_traced(x, Wq, Wk, Wv, Wo):
    """Run with NTFF tracing; returns (full_output, BassKernelResults)."""
    res = _run_device(np.asarray(x, np.float32), np.asarray(Wq, np.float32),
                      np.asarray(Wk, np.float32), np.asarray(Wv, np.float32),
                      np.asarray(Wo, np.float32), trace=True)
    out = np.empty((B, S, D), dtype=np.float32)
    for b in range(B):
        acc = res.results[b * CORES_PER_BATCH]["out"]
        for i in range(1, CORES_PER_BATCH):
            acc = acc + res.results[b * CORES_PER_BATCH + i]["out"]
        out[b] = acc
    return out, res



# revision 2
# speedup vs baseline: 1.1907x; 1.1907x over previous
"""Causal self-attention (B=2, S=2048, D=2048, H=16, HD=128) on 8 TRN2 cores.

Sharding: core c -> batch b = c//4, heads 4*(c%4)..4*(c%4)+3 (tensor-parallel
over heads within a batch; data-parallel over batch across core groups).

Fused single-pass design, fully SBUF-resident (no DRAM bounce):
  for each s-block sb of 512:
    - Q^T/K^T (RoPE applied) and V projections for the 4 local heads, written
      straight into SBUF homes in bf16,
    - causal attention for q-block qb==sb over k-chunks 0..4*sb+3 in
      transposed-score layout (S^T[k,q]); exp on ACT, row-sums l via a
      128-wide ones matmul (output arrives pre-broadcast over partitions),
      PV produces ctx^T[hd,q],
    - output projection for qb==sb interleaved into the next s-block's
      projection groups; partial [2048,2048] summed on host across 4 cores.
All PE operands are bf16 (full PE rate, no <256-col fp32r penalty, half the
DMA/SBUF traffic); PSUM accumulation stays fp32.
"""

import math
from collections import deque

import numpy as np
import ml_dtypes

import concourse.bacc as bacc
import concourse.mybir as mybir
from concourse.tile import TileContext
from concourse.bass_utils import run_bass_kernel_spmd

B, S, D = 2, 2048, 2048
H, HD = 16, 128
ROPE_THETA = 10000.0

N_CORES = 8
CORES_PER_BATCH = 4
HPC = H // (N_CORES // B)  # heads per core = 4
HL = HPC * HD              # 512 local head-dim columns
NDC = D // 128             # 16 contraction chunks
NSB = S // 512             # 4 s-blocks

F32 = mybir.dt.float32
BF16 = mybir.dt.bfloat16
AF = mybir.ActivationFunctionType
NPBF = ml_dtypes.bfloat16


def _mm(nc, out, lhsT, rhs, start, stop):
    nc.tensor.matmul(out, lhsT, rhs, start=start, stop=stop)


def _build():
    nc = bacc.Bacc("TRN2", target_bir_lowering=False, debug=False)

    xT = nc.dram_tensor("xT", [D, S], BF16, kind="ExternalInput")
    wq = nc.dram_tensor("wq", [D, HL], BF16, kind="ExternalInput")
    wk = nc.dram_tensor("wk", [D, HL], BF16, kind="ExternalInput")
    wv = nc.dram_tensor("wv", [D, HL], BF16, kind="ExternalInput")
    wo = nc.dram_tensor("wo", [HL, D], BF16, kind="ExternalInput")
    cosT = nc.dram_tensor("cosT", [HD, S], F32, kind="ExternalInput")
    sinT = nc.dram_tensor("sinT", [HD, S], F32, kind="ExternalInput")
    pmatT = nc.dram_tensor("pmatT", [HD, HD], BF16, kind="ExternalInput")
    maskT = nc.dram_tensor("maskT", [128, 512], BF16, kind="ExternalInput")
    onesd = nc.dram_tensor("onesd", [128, 128], BF16, kind="ExternalInput")
    out = nc.dram_tensor("out", [S, D], F32, kind="ExternalOutput")

    with TileContext(nc) as tc:
        with (
            tc.tile_pool(name="homes", bufs=1) as homes,
            tc.tile_pool(name="consts", bufs=1) as consts,
            tc.tile_pool(name="wpool", bufs=1) as wpool,
            tc.tile_pool(name="xtp", bufs=18) as xtp,
            tc.tile_pool(name="st1", bufs=3) as st1,
            tc.tile_pool(name="ptp", bufs=6) as ptp,
            tc.tile_pool(name="smp", bufs=2) as smp,
            tc.tile_pool(name="outp", bufs=3) as outp,
            tc.tile_pool(name="psA", bufs=3, space="PSUM") as psA,
            tc.tile_pool(name="psB", bufs=3, space="PSUM") as psB,
            tc.tile_pool(name="psC", bufs=2, space="PSUM") as psC,
        ):
            # persistent SBUF homes (bf16)
            qh = [[homes.tile([HD, 512], BF16, name=f"qh{h}_{sb}")
                   for sb in range(NSB)] for h in range(HPC)]
            kh = [[homes.tile([HD, 512], BF16, name=f"kh{h}_{sb}")
                   for sb in range(NSB)] for h in range(HPC)]
            vh = [homes.tile([128, HL], BF16, name=f"vh{kc}")
                  for kc in range(4 * NSB)]
            ch = [[homes.tile([HD, 512], BF16, name=f"ch{h}_{sb}")
                   for sb in range(NSB)] for h in range(HPC)]

            pmat_sb = consts.tile([HD, HD], BF16, name="pmat_sb")
            mask_sb = consts.tile([128, 512], BF16, name="mask_sb")
            ones_sb = consts.tile([128, 128], BF16, name="ones_sb")
            cos_sb = consts.tile([HD, S], F32, name="cos_sb")
            sin_sb = consts.tile([HD, S], F32, name="sin_sb")
            wo_sb = wpool.tile([128, HPC, D], BF16, name="wo_sb")

            def load_w_chunk(w_d, dc, tag):
                wt = wpool.tile([128, HL], BF16, tag=f"{tag}{dc}", name="wt")
                nc.sync.dma_start(out=wt[:], in_=w_d[dc * 128:(dc + 1) * 128, :])
                return wt

            def load_xt_chunk(dc, sb):
                xt = xtp.tile([128, 512], BF16, tag="xt", name="xt")
                nc.sync.dma_start(
                    out=xt[:], in_=xT[dc * 128:(dc + 1) * 128,
                                      sb * 512:(sb + 1) * 512])
                return xt

            # -------- prologue DMAs: interleave x(sb=0) with wq chunks ------
            xts = []
            wq_t, wk_t, wv_t = [], [], []
            for dc in range(NDC):
                xts.append(load_xt_chunk(dc, 0))
                wq_t.append(load_w_chunk(wq, dc, "wq"))
                if dc == 3:
                    nc.sync.dma_start(out=pmat_sb[:], in_=pmatT[:])
                    nc.sync.dma_start(out=mask_sb[:], in_=maskT[:])
                    nc.sync.dma_start(out=ones_sb[:], in_=onesd[:])
                    nc.sync.dma_start(out=cos_sb[:], in_=cosT[:])
                    nc.sync.dma_start(out=sin_sb[:], in_=sinT[:])
            for dc in range(NDC):
                wk_t.append(load_w_chunk(wk, dc, "wk"))
            for dc in range(NDC):
                wv_t.append(load_w_chunk(wv, dc, "wv"))
            nc.sync.dma_start(out=wo_sb[:],
                              in_=wo.rearrange("(r p) c -> p r c", p=128))

            # proj finishers, lagged one group behind emission
            finishers = deque()

            def emit_finisher():
                kind, args = finishers.popleft()
                if kind == "qk":
                    ps, dst, sl = args
                    qraw = st1.tile([128, 512], BF16, tag="qraw", name="qraw")
                    nc.scalar.activation(qraw[:], ps[:], AF.Copy)
                    rot = psB.tile([128, 512], F32, tag="b", name="rot")
                    _mm(nc, rot[:], pmat_sb[:], qraw[:], start=True, stop=True)
                    acos = st1.tile([128, 512], F32, tag="acos", name="acos")
                    nc.vector.tensor_mul(acos[:], ps[:], cos_sb[:, sl])
                    rsin = st1.tile([128, 512], F32, tag="rsin", name="rsin")
                    nc.vector.tensor_mul(rsin[:], rot[:], sin_sb[:, sl])
                    nc.vector.tensor_add(dst[:], rsin[:], acos[:])
                else:
                    ps, kc = args
                    nc.scalar.activation(vh[kc][:], ps[:], AF.Copy)

            # attention bookkeeping
            lagq = deque()    # (lps, pv, pt, vtc, ncols, first, last)
            fin = deque()     # (h, sb, lps, pv)
            lpv_done = set()

            def emit_lpv(job):
                lps, pv, pt, vtc, ncols, first, last = job
                _mm(nc, lps[:, 512 - ncols:], ones_sb[:], pt[:, :ncols],
                    start=first, stop=last)
                _mm(nc, pv[:, 512 - ncols:], vtc, pt[:, :ncols],
                    start=first, stop=last)
                if last:
                    lpv_done.add(id(lps))

            def emit_finalize(job):
                h, sb, lps, pv = job
                rcps = smp.tile([128, 512], F32, tag="rcps", name="rcps")
                rcp = smp.tile([128, 512], F32, tag="rcp", name="rcp")
                nc.vector.reciprocal_approx_accurate(rcp[:], lps[:], rcps[:])
                nc.vector.tensor_mul(ch[h][sb][:], pv[:], rcp[:])

            def pop_finalize():
                if fin and id(fin[0][2]) in lpv_done:
                    emit_finalize(fin.popleft())

            outproj_q = deque()   # (qc, db) pending output-projection groups

            def emit_outproj():
                qc, db = outproj_q.popleft()
                sb = qc // 4
                ops = psA.tile([128, 512], F32, tag="a", name="ops")
                for h in range(HPC):
                    _mm(nc, ops[:],
                        ch[h][sb][:, (qc % 4) * 128:(qc % 4 + 1) * 128],
                        wo_sb[:, h, db * 512:(db + 1) * 512],
                        start=(h == 0), stop=(h == HPC - 1))
                osb = outp.tile([128, 512], F32, tag="osb", name="osb")
                nc.scalar.activation(osb[:], ops[:], AF.Copy)
                nc.sync.dma_start(
                    out=out[qc * 128:(qc + 1) * 128, db * 512:(db + 1) * 512],
                    in_=osb[:])

            # ------------------------- main fused loop ----------------------
            for sb in range(NSB):
                sl = slice(sb * 512, (sb + 1) * 512)
                if sb > 0:
                    xts = [load_xt_chunk(dc, sb) for dc in range(NDC)]

                # projection groups: Q heads, K heads, then V s-chunks;
                # interleave previous s-block's outproj groups between them.
                for w_t, dsts in ((wq_t, qh), (wk_t, kh)):
                    for h in range(HPC):
                        ps = psA.tile([128, 512], F32, tag="a", name="ps")
                        for dc in range(NDC):
                            _mm(nc, ps[:],
                                w_t[dc][:, h * HD:(h + 1) * HD],
                                xts[dc][:],
                                start=(dc == 0), stop=(dc == NDC - 1))
                        finishers.append(("qk", (ps, dsts[h][sb], sl)))
                        if len(finishers) > 1:
                            emit_finisher()
                        for _ in range(2):
                            if outproj_q:
                                emit_outproj()
                for sc in range(4):
                    ps = psA.tile([128, 512], F32, tag="a", name="ps")
                    for dc in range(NDC):
                        _mm(nc, ps[:],
                            xts[dc][:, sc * 128:(sc + 1) * 128],
                            wv_t[dc][:],
                            start=(dc == 0), stop=(dc == NDC - 1))
                    finishers.append(("v", (ps, 4 * sb + sc)))
                    if len(finishers) > 1:
                        emit_finisher()
                    for _ in range(2):
                        if outproj_q:
                            emit_outproj()
                while finishers:
                    emit_finisher()
                while outproj_q:
                    emit_outproj()

                # ------- attention for q-block qb == sb, all local heads ----
                nk = 4 * sb + 4
                for h in range(HPC):
                    lps = psC.tile([128, 512], F32, tag="c", name="lps")
                    pv = psB.tile([128, 512], F32, tag="b", name="pv")
                    for kc in range(nk):
                        j = kc - 4 * sb
                        ncols = 512 if j < 0 else 512 - 128 * j
                        sps = psA.tile([128, 512], F32, tag="a", name="sps")
                        _mm(nc, sps[:, :ncols],
                            kh[h][kc // 4][:, (kc % 4) * 128:(kc % 4 + 1) * 128],
                            qh[h][sb][:, 512 - ncols:],
                            start=True, stop=True)
                        pt = ptp.tile([128, 512], BF16, tag="pt", name="pt")
                        nc.scalar.activation(pt[:, :ncols], sps[:, :ncols], AF.Exp)
                        if j >= 0:
                            nc.vector.tensor_mul(pt[:, :ncols], pt[:, :ncols],
                                                 mask_sb[:, :ncols])
                        lagq.append((lps, pv, pt,
                                     vh[kc][:, h * HD:(h + 1) * HD],
                                     ncols, kc == 0, kc == nk - 1))
                        while len(lagq) > 2:
                            emit_lpv(lagq.popleft())
                        if kc % 2 == 1:
                            pop_finalize()
                    fin.append((h, sb, lps, pv))
                while lagq:
                    emit_lpv(lagq.popleft())
                while fin:
                    pop_finalize()
                # queue this s-block's output projection
                for qc in range(4 * sb, 4 * sb + 4):
                    for db in range(D // 512):
                        outproj_q.append((qc, db))
            while outproj_q:
                emit_outproj()

    nc.compile()
    return nc


_NC_CACHE = None


def _get_nc():
    global _NC_CACHE
    if _NC_CACHE is None:
        _NC_CACHE = _build()
    return _NC_CACHE


def _host_tables():
    # Replicate reference RoPE tables in float32 arithmetic, transposed.
    inv_freq = np.float32(1.0) / np.power(
        np.float32(ROPE_THETA), np.arange(0, HD, 2).astype(np.float32) / np.float32(HD)
    )
    pos = np.arange(S, dtype=np.float32)
    freqs = pos[:, None] * inv_freq[None, :]
    angles = np.concatenate([freqs, freqs], axis=1)  # [S, HD]
    cos_t = np.ascontiguousarray(np.cos(angles).astype(np.float32).T)  # [HD, S]
    sin_t = np.ascontiguousarray(np.sin(angles).astype(np.float32).T)
    # rotate_half as a left-multiply matrix P: (P q)[2i] = -q[2i+1], [2i+1] = q[2i].
    # matmul computes lhsT.T @ rhs, so feed P.T.
    pmat = np.zeros((HD, HD), dtype=np.float32)
    for i in range(HD // 2):
        pmat[2 * i, 2 * i + 1] = -1.0
        pmat[2 * i + 1, 2 * i] = 1.0
    pmat_t = np.ascontiguousarray(pmat.T).astype(NPBF)
    mask = (np.arange(128)[:, None] <= np.arange(512)[None, :]).astype(NPBF)
    return cos_t, sin_t, pmat_t, mask


_ONES = np.ones((128, 128), dtype=NPBF)


def kernel(x, Wq, Wk, Wv, Wo):
    x = np.asarray(x, dtype=np.float32)
    Wq = np.asarray(Wq, dtype=np.float32)
    Wk = np.asarray(Wk, dtype=np.float32)
    Wv = np.asarray(Wv, dtype=np.float32)
    Wo = np.asarray(Wo, dtype=np.float32)

    results = _run_device(x, Wq, Wk, Wv, Wo)

    out = np.empty((B, S, D), dtype=np.float32)
    for b in range(B):
        acc = results[b * CORES_PER_BATCH]["out"]
        for i in range(1, CORES_PER_BATCH):
            acc = acc + results[b * CORES_PER_BATCH + i]["out"]
        out[b] = acc
    return out


def _make_in_maps(x, Wq, Wk, Wv, Wo):
    cos_t, sin_t, pmat_t, mask = _host_tables()
    scale = np.float32(1.0 / math.sqrt(HD))
    wq_scaled = (Wq * scale).astype(np.float32)
    xTb = [np.ascontiguousarray(x[b].T).astype(NPBF) for b in range(B)]
    in_maps = []
    for c in range(N_CORES):
        b = c // CORES_PER_BATCH
        g = c % CORES_PER_BATCH
        hs = slice(g * HL, (g + 1) * HL)
        in_maps.append({
            "xT": xTb[b],
            "wq": np.ascontiguousarray(wq_scaled[:, hs]).astype(NPBF),
            "wk": np.ascontiguousarray(Wk[:, hs]).astype(NPBF),
            "wv": np.ascontiguousarray(Wv[:, hs]).astype(NPBF),
            "wo": np.ascontiguousarray(Wo[hs, :]).astype(NPBF),
            "cosT": cos_t,
            "sinT": sin_t,
            "pmatT": pmat_t,
            "maskT": mask,
            "onesd": _ONES,
        })
    return in_maps


def _run_device(x, Wq, Wk, Wv, Wo, trace=False):
    nc = _get_nc()
    in_maps = _make_in_maps(x, Wq, Wk, Wv, Wo)
    res = run_bass_kernel_spmd(nc, in_maps, core_ids=list(range(N_CORES)), trace=trace)
    if trace:
        return res
    return res.results


def run_traced(x, Wq, Wk, Wv, Wo):
    """Run with NTFF tracing; returns (full_output, BassKernelResults)."""
    res = _run_device(np.asarray(x, np.float32), np.asarray(Wq, np.float32),
                      np.asarray(Wk, np.float32), np.asarray(Wv, np.float32),
                      np.asarray(Wo, np.float32), trace=True)
    out = np.empty((B, S, D), dtype=np.float32)
    for b in range(B):
        acc = res.results[b * CORES_PER_BATCH]["out"]
        for i in range(1, CORES_PER_BATCH):
            acc = acc + res.results[b * CORES_PER_BATCH + i]["out"]
        out[b] = acc
    return out, res
